# revision 1
# baseline (speedup 1.0000x reference)
"""Trainium2 Bass kernel for nn_MAE_CalcLoss_Raw (masked MSE loss).

reference math:
    masked   = mean_b[ mean_{i,d} (outputs[b, mask_id[b,i], d]   - orig[b, mask_id[b,i], d])^2 ]
    unmasked = mean_b[ mean_{i,d} (outputs[b, unmask_id[b,i], d] - orig[b, unmask_id[b,i], d])^2 ]
    loss = masked + 0.1 * unmasked

Rewrite: gathering rows by index (with repeats) is a weighted sum over
referenced (b, s) rows.  With cnt_m[b,s] = #occurrences of s in
mask_id[b], cnt_u likewise:

    loss = sum_{b,s} w[b,s] * ||outputs[b,s,:] - orig[b,s,:]||^2
    w[b,s] = cnt_m[b,s]/(B*Nm*D) + ALPHA*cnt_u[b,s]/(B*Nu*D)

The device streams both full tensors at the per-core DMA ceiling
(~387 GB/s measured): per half-tile, two 1 MB loads, DVE subtract in
place, then square + per-row accumulate split across ACT (3 groups)
and DVE (1 group) into a [128, 128] accumulator, which is DMA'd out
raw; the host applies the histogram weights in float64.  Data-parallel
over B: 8 samples per core on 8 cores.

A row-gather variant (indirect DMA over the ~64% of rows actually
referenced) is kept behind USE_GATHER but disabled: SWDGE descriptor
generation costs ~10 ns/row, capping gather at ~230 GB/s equivalent —
slower than just streaming everything.
"""

import numpy as np

ALPHA = 0.1
B, S, D = 64, 2048, 512
NM, NU = 1536, 512
N_CORES = 8
BPC = B // N_CORES            # samples per core
R = BPC * S                   # rows per core = 16384
GROUPS = 8                    # 128-row groups per tile
TILE_ROWS = GROUPS * 128      # 1024 rows per tile (2 MB per tensor)

N_TILES_FULL = R // TILE_ROWS          # 16
N_TILES_GATH = 11                      # 11264 gathered rows (max seen 10445)
NIDX = N_TILES_GATH * TILE_ROWS        # 11264
USE_GATHER = False

_CACHE: dict = {}


def _build_nc(gather: bool):
    import concourse.bacc as bacc
    import concourse.bass as bass
    import concourse.tile as tile
    import concourse.mybir as mybir

    f32 = mybir.dt.float32
    n_tiles = N_TILES_GATH if gather else N_TILES_FULL
    ncol = n_tiles * GROUPS
    nc = bacc.Bacc(
        "TRN2",
        target_bir_lowering=False,
        debug=False,
        enable_asserts=False,
        num_devices=N_CORES,
    )
    x_d = nc.dram_tensor("x", [R, D], f32, kind="ExternalInput").ap()
    y_d = nc.dram_tensor("y", [R, D], f32, kind="ExternalInput").ap()
    if gather:
        idx_d = nc.dram_tensor(
            "idx", [128, ncol], mybir.dt.int32, kind="ExternalInput"
        ).ap()
    p_d = nc.dram_tensor("racc_out", [128, ncol], f32, kind="ExternalOutput").ap()

    with tile.TileContext(nc) as tc:
        with (
            tc.tile_pool(name="io", bufs=4) as io,
            tc.tile_pool(name="acc", bufs=1) as acc,
        ):
            if gather:
                idx_sb = acc.tile([128, ncol], mybir.dt.int32, tag="idx")
                nc.sync.dma_start(idx_sb[:], idx_d[:])
            racc = acc.tile([128, ncol], f32, tag="racc")

            HG = GROUPS // 2  # half-tile: 4 groups, 1 MB per tensor
            n_halves = 2 * n_tiles
            for h in range(n_halves):
                if not gather and h == n_halves - 1:
                    # final half-tile in single-group chunks: shortens the
                    # compute tail after the last DMA lands
                    for g in range(HG):
                        j = h * HG + g
                        xg = io.tile([128, 1, D], f32, tag="xf")
                        nc.sync.dma_start(
                            xg[:],
                            x_d[bass.ts(j, 128), :].rearrange(
                                "(g p) d -> p g d", g=1, p=128
                            ),
                        )
                        yg = io.tile([128, 1, D], f32, tag="yf")
                        nc.sync.dma_start(
                            yg[:],
                            y_d[bass.ts(j, 128), :].rearrange(
                                "(g p) d -> p g d", g=1, p=128
                            ),
                        )
                        nc.vector.tensor_sub(xg[:], xg[:], yg[:])
                        if g == HG - 1:
                            nc.vector.scalar_tensor_tensor(
                                out=xg[:, 0, :],
                                in0=xg[:, 0, :],
                                scalar=1.0,
                                in1=xg[:, 0, :],
                                op0=mybir.AluOpType.mult,
                                op1=mybir.AluOpType.mult,
                                accum_out=racc[:, j : j + 1],
                            )
                        else:
                            nc.scalar.activation(
                                xg[:, 0, :],
                                xg[:, 0, :],
                                mybir.ActivationFunctionType.Square,
                                accum_out=racc[:, j : j + 1],
                            )
                    continue
                xt = io.tile([128, HG, D], f32, tag="x")
                yt = io.tile([128, HG, D], f32, tag="y")
                if gather:
                    # HW SWDGE walks only single-column offset APs correctly
                    for g in range(HG):
                        j = h * HG + g
                        nc.gpsimd.indirect_dma_start(
                            out=xt[:, g, :],
                            out_offset=None,
                            in_=x_d[:],
                            in_offset=bass.IndirectOffsetOnAxis(
                                ap=idx_sb[:, j : j + 1], axis=0
                            ),
                        )
                        nc.gpsimd.indirect_dma_start(
                            out=yt[:, g, :],
                            out_offset=None,
                            in_=y_d[:],
                            in_offset=bass.IndirectOffsetOnAxis(
                                ap=idx_sb[:, j : j + 1], axis=0
                            ),
                        )
                else:
                    nc.sync.dma_start(
                        xt[:],
                        x_d[bass.ts(h, HG * 128), :].rearrange(
                            "(g p) d -> p g d", g=HG, p=128
                        ),
                    )
                    nc.sync.dma_start(
                        yt[:],
                        y_d[bass.ts(h, HG * 128), :].rearrange(
                            "(g p) d -> p g d", g=HG, p=128
                        ),
                    )
                # diff in place on DVE
                nc.vector.tensor_sub(xt[:], xt[:], yt[:])
                # square + per-row accumulate: 3 groups on ACT, 1 on DVE
                for g in range(HG):
                    j = h * HG + g
                    if g == HG - 1:
                        nc.vector.scalar_tensor_tensor(
                            out=xt[:, g, :],
                            in0=xt[:, g, :],
                            scalar=1.0,
                            in1=xt[:, g, :],
                            op0=mybir.AluOpType.mult,
                            op1=mybir.AluOpType.mult,
                            accum_out=racc[:, j : j + 1],
                        )
                    else:
                        nc.scalar.activation(
                            xt[:, g, :],
                            xt[:, g, :],
                            mybir.ActivationFunctionType.Square,
                            accum_out=racc[:, j : j + 1],
                        )

            nc.sync.dma_start(p_d[:], racc[:])

    nc.compile()
    return nc


def _get_nc(gather: bool):
    key = "gather" if gather else "full"
    if key not in _CACHE:
        _CACHE[key] = _build_nc(gather)
    return _CACHE[key]


def _hists(mask_id, unmask_id):
    rows = np.arange(B)[:, None]
    cm = np.zeros((B, S), np.float64)
    np.add.at(cm, (rows, mask_id.astype(np.int64)), 1.0)
    cu = np.zeros((B, S), np.float64)
    np.add.at(cu, (rows, unmask_id.astype(np.int64)), 1.0)
    return cm, cu


def _in_maps(outputs, orig_image, mask_id, unmask_id, force_full: bool = False):
    """Returns (maps, gather_flag)."""
    cm, cu = _hists(np.asarray(mask_id), np.asarray(unmask_id))
    w = cm / (B * NM * D) + ALPHA * cu / (B * NU * D)  # [B,S] f64
    ref = (cm + cu) > 0                                # referenced rows

    x = np.ascontiguousarray(np.asarray(outputs, dtype=np.float32)).reshape(B * S, D)
    y = np.ascontiguousarray(np.asarray(orig_image, dtype=np.float32)).reshape(B * S, D)

    # Device-side row gather measured slower than full streaming on this HW
    # (SWDGE descriptor gen ~10 ns/row caps gather at ~230 GB/s equivalent
    # vs 341 GB/s streamed), so the gather path stays disabled.
    counts = ref.reshape(N_CORES, BPC * S).sum(axis=1)
    gather = USE_GATHER and bool(counts.max() <= NIDX) and not force_full

    maps = []
    wmats = []
    for c in range(N_CORES):
        m = {
            "x": x[c * R : (c + 1) * R],
            "y": y[c * R : (c + 1) * R],
        }
        if gather:
            refs = np.nonzero(ref[c * BPC : (c + 1) * BPC].reshape(R))[0]
            L = np.zeros(NIDX, np.int64)
            L[: len(refs)] = refs
            wL = np.zeros(NIDX, np.float64)
            wL[: len(refs)] = w[c * BPC : (c + 1) * BPC].reshape(R)[refs]
            # tile i gathers rows L[i*1024 + p*8 + g] -> dest[p, g, :]
            m["idx"] = np.ascontiguousarray(
                L.reshape(N_TILES_GATH, 128, GROUPS)
                .transpose(1, 0, 2)
                .reshape(128, N_TILES_GATH * GROUPS)
                .astype(np.int32)
            )
            wmats.append(
                wL.reshape(N_TILES_GATH, 128, GROUPS)
                .transpose(1, 0, 2)
                .reshape(128, N_TILES_GATH * GROUPS)
            )
        else:
            w_c = w[c * BPC : (c + 1) * BPC].reshape(R)
            wmats.append(
                w_c.reshape(N_TILES_FULL, GROUPS, 128)
                .transpose(2, 0, 1)
                .reshape(128, N_TILES_FULL * GROUPS)
            )
        maps.append(m)
    return maps, gather, wmats


def _run(inputs: dict, trace: bool = False, force_full: bool = False, **kw):
    from concourse.bass_utils import run_bass_kernel_spmd

    maps, gather, wmats = _in_maps(**inputs, force_full=force_full)
    nc = _get_nc(gather)
    res = run_bass_kernel_spmd(nc, maps, list(range(N_CORES)), trace=trace, **kw)
    total = np.float64(0.0)
    for c in range(N_CORES):
        racc = np.asarray(res.results[c]["racc_out"], dtype=np.float64)
        total += (racc * wmats[c]).sum()
    return np.asarray(total, dtype=np.float32), res


def kernel(outputs, orig_image, mask_id, unmask_id):
    outputs = np.asarray(outputs)
    orig_image = np.asarray(orig_image)
    mask_id = np.asarray(mask_id)
    unmask_id = np.asarray(unmask_id)
    assert outputs.shape == (B, S, D), outputs.shape
    assert orig_image.shape == (B, S, D), orig_image.shape
    assert mask_id.shape == (B, NM), mask_id.shape
    assert unmask_id.shape == (B, NU), unmask_id.shape
    out, _ = _run(
        {
            "outputs": outputs,
            "orig_image": orig_image,
            "mask_id": mask_id,
            "unmask_id": unmask_id,
        }
    )
    return out



# revision 8
# speedup vs baseline: 1.0661x; 1.0661x over previous
"""Trainium2 Bass kernel for nn_MAE_CalcLoss_Raw (masked MSE loss).

reference math:
    masked   = mean_b[ mean_{i,d} (outputs[b, mask_id[b,i], d]   - orig[b, mask_id[b,i], d])^2 ]
    unmasked = mean_b[ mean_{i,d} (outputs[b, unmask_id[b,i], d] - orig[b, unmask_id[b,i], d])^2 ]
    loss = masked + 0.1 * unmasked

Rewrite: gathering rows by index (with repeats) is a weighted sum over
referenced (b, s) rows.  With cnt_m[b,s] = #occurrences of s in
mask_id[b], cnt_u likewise:

    loss = sum_{b,s} w[b,s] * ||outputs[b,s,:] - orig[b,s,:]||^2
    w[b,s] = cnt_m[b,s]/(B*Nm*D) + ALPHA*cnt_u[b,s]/(B*Nu*D)

Only ~63% of rows are referenced (2048 draws with replacement from 2048
rows -> 1-1/e distinct), so instead of streaming both tensors in full
(HBM-bound at ~358 GB/s/core = ~187 us) the kernel gathers just the
referenced rows (~42 MB/core -> ~120 us floor).

The gather uses the InstDMAGatherAnt custom GPSIMD instruction.  Its
Q7 descriptor generation costs ~8.8 ns/index and is serialized on the
Pool engine, so per-row gathers (~10.4k rows x 2 tensors/core) would be
Pool-bound at ~185 us.  Runs of consecutive referenced rows are instead
decomposed exactly into windows of {8,4,2,1} rows (one descriptor per
window, elem_step=512 < elem_size allows windows at arbitrary row
offsets via a manually-built overlapping access pattern).  ~5.05k
windows/tensor/core -> Pool ~90 us, hidden under the ~120 us DMA.

Per chunk (1024 gathered rows, 2 MB/tensor): gather x, gather y (Pool),
subtract in place (DVE), then 8 square+per-row-accumulate ops (6 on
ACT, 2 on DVE) into a [128, 104] accumulator DMA'd out raw; the host
applies the per-row histogram weights in float64 (pad slots are masked
out by weight==0).  Data-parallel over B: 8 samples per core.

If a window class overflows its compiled capacity (won't happen for
this input distribution; margins are >5 sigma), the kernel falls back
to the previous full-streaming variant which is always correct.
"""

import numpy as np

ALPHA = 0.1
B, S, D = 64, 2048, 512
NM, NU = 1536, 512
N_CORES = 8
BPC = B // N_CORES            # samples per core
R = BPC * S                   # rows per core = 16384

# --- gather-kernel geometry ---
# (window_rows, n_chunks); chunk = 1024 gathered rows -> slots/chunk = 1024//w
CLASSES = [(8, 2), (4, 4), (2, 4), (1, 3)]
import os as _os
if _os.environ.get("K_CLASSES"):
    CLASSES = [
        (int(p.split(":")[0]), int(p.split(":")[1]))
        for p in _os.environ["K_CLASSES"].split(",")
    ]
ROWS_PER_CHUNK = 1024
N_CHUNKS = sum(n for _, n in CLASSES)             # 13
NCOL = N_CHUNKS * (ROWS_PER_CHUNK // 128)         # 104 racc columns
IDXCOL = sum(n * (ROWS_PER_CHUNK // w) // 16 for w, n in CLASSES)  # 400
CAPS = {w: n * (ROWS_PER_CHUNK // w) for w, n in CLASSES}
ACT_COLS = 6                  # of the 8 per-chunk columns, 6 on ACT, 2 on DVE

# --- streaming-kernel geometry (fallback) ---
GROUPS = 8                    # 128-row groups per tile
TILE_ROWS = GROUPS * 128      # 1024 rows per tile (2 MB per tensor)
N_TILES_FULL = R // TILE_ROWS          # 16

_CACHE: dict = {}


def _build_gather_nc():
    import concourse.bacc as bacc
    import concourse.tile as tile
    import concourse.mybir as mybir
    import bass_rust

    f32 = mybir.dt.float32
    i16 = mybir.dt.int16

    nc = bacc.Bacc(
        "TRN2",
        target_bir_lowering=False,
        debug=False,
        enable_asserts=False,
        num_devices=N_CORES,
    )
    x_d = nc.dram_tensor("x", [R, D], f32, kind="ExternalInput").ap()
    y_d = nc.dram_tensor("y", [R, D], f32, kind="ExternalInput").ap()
    idx_d = nc.dram_tensor("idx", [128, IDXCOL], i16, kind="ExternalInput").ap()
    p_d = nc.dram_tensor("racc_out", [128, NCOL], f32, kind="ExternalOutput").ap()

    # Overlapping window views: row-stride 512 elems, window length w*512.
    def win_view(base, w):
        if w == 1:
            return base
        v = base.copy()
        v.ap = bass_rust.VecI64Pair([[D, R - w + 1], [1, w * D]])
        return v

    xv = {w: win_view(x_d, w) for w, _ in CLASSES}
    yv = {w: win_view(y_d, w) for w, _ in CLASSES}

    with tile.TileContext(nc) as tc:
        with (
            tc.tile_pool(name="io", bufs=int(_os.environ.get("K_BUFS", "4"))) as io,
            tc.tile_pool(name="acc", bufs=1) as acc,
        ):
            idx_sb = acc.tile([128, IDXCOL], i16, tag="idx")
            nc.sync.dma_start(idx_sb[:], idx_d[:])
            racc = acc.tile([128, NCOL], f32, tag="racc")

            icol = 0
            rcol = 0
            for w, nch in CLASSES:
                cs = ROWS_PER_CHUNK // w       # slots per chunk
                ccols = cs // 128              # tile columns
                icols = cs // 16               # idx columns per chunk
                for _ in range(nch):
                    tg = f"{w}" if _os.environ.get("K_TAGS") else ""
                    xt = io.tile([128, ccols, w * D], f32, tag="x" + tg)
                    yt = io.tile([128, ccols, w * D], f32, tag="y" + tg)
                    ixap = idx_sb[:, icol:icol + icols]
                    step = None if w == 1 else D
                    nc.gpsimd.dma_gather(
                        xt[:], xv[w], ixap, cs, cs, w * D, elem_step=step)
                    nc.gpsimd.dma_gather(
                        yt[:], yv[w], ixap, cs, cs, w * D, elem_step=step)
                    nc.vector.tensor_sub(xt[:], xt[:], yt[:])
                    for g in range(ROWS_PER_CHUNK // 128):   # 8 cols
                        c, r = divmod(g, w)
                        src = xt[:, c, r * D:(r + 1) * D]
                        col = racc[:, rcol + g:rcol + g + 1]
                        if g < ACT_COLS:
                            nc.scalar.activation(
                                src, src,
                                mybir.ActivationFunctionType.Square,
                                accum_out=col)
                        else:
                            nc.vector.scalar_tensor_tensor(
                                out=src, in0=src, scalar=1.0, in1=src,
                                op0=mybir.AluOpType.mult,
                                op1=mybir.AluOpType.mult,
                                accum_out=col)
                    icol += icols
                    rcol += ROWS_PER_CHUNK // 128

            nc.sync.dma_start(p_d[:], racc[:])

    nc.compile()
    return nc


def _build_stream_nc():
    import concourse.bacc as bacc
    import concourse.bass as bass
    import concourse.tile as tile
    import concourse.mybir as mybir

    f32 = mybir.dt.float32
    ncol = N_TILES_FULL * GROUPS
    nc = bacc.Bacc(
        "TRN2",
        target_bir_lowering=False,
        debug=False,
        enable_asserts=False,
        num_devices=N_CORES,
    )
    x_d = nc.dram_tensor("x", [R, D], f32, kind="ExternalInput").ap()
    y_d = nc.dram_tensor("y", [R, D], f32, kind="ExternalInput").ap()
    p_d = nc.dram_tensor("racc_out", [128, ncol], f32, kind="ExternalOutput").ap()

    with tile.TileContext(nc) as tc:
        with (
            tc.tile_pool(name="io", bufs=4) as io,
            tc.tile_pool(name="acc", bufs=1) as acc,
        ):
            racc = acc.tile([128, ncol], f32, tag="racc")

            HG = GROUPS // 2  # half-tile: 4 groups, 1 MB per tensor
            n_halves = 2 * N_TILES_FULL
            for h in range(n_halves):
                if h == n_halves - 1:
                    # final half-tile in single-group chunks: shortens the
                    # compute tail after the last DMA lands
                    for g in range(HG):
                        j = h * HG + g
                        xg = io.tile([128, 1, D], f32, tag="xf")
                        nc.sync.dma_start(
                            xg[:],
                            x_d[bass.ts(j, 128), :].rearrange(
                                "(g p) d -> p g d", g=1, p=128
                            ),
                        )
                        yg = io.tile([128, 1, D], f32, tag="yf")
                        nc.sync.dma_start(
                            yg[:],
                            y_d[bass.ts(j, 128), :].rearrange(
                                "(g p) d -> p g d", g=1, p=128
                            ),
                        )
                        nc.vector.tensor_sub(xg[:], xg[:], yg[:])
                        if g == HG - 1:
                            nc.vector.scalar_tensor_tensor(
                                out=xg[:, 0, :],
                                in0=xg[:, 0, :],
                                scalar=1.0,
                                in1=xg[:, 0, :],
                                op0=mybir.AluOpType.mult,
                                op1=mybir.AluOpType.mult,
                                accum_out=racc[:, j : j + 1],
                            )
                        else:
                            nc.scalar.activation(
                                xg[:, 0, :],
                                xg[:, 0, :],
                                mybir.ActivationFunctionType.Square,
                                accum_out=racc[:, j : j + 1],
                            )
                    continue
                xt = io.tile([128, HG, D], f32, tag="x")
                yt = io.tile([128, HG, D], f32, tag="y")
                nc.sync.dma_start(
                    xt[:],
                    x_d[bass.ts(h, HG * 128), :].rearrange(
                        "(g p) d -> p g d", g=HG, p=128
                    ),
                )
                nc.sync.dma_start(
                    yt[:],
                    y_d[bass.ts(h, HG * 128), :].rearrange(
                        "(g p) d -> p g d", g=HG, p=128
                    ),
                )
                # diff in place on DVE
                nc.vector.tensor_sub(xt[:], xt[:], yt[:])
                # square + per-row accumulate: 3 groups on ACT, 1 on DVE
                for g in range(HG):
                    j = h * HG + g
                    if g == HG - 1:
                        nc.vector.scalar_tensor_tensor(
                            out=xt[:, g, :],
                            in0=xt[:, g, :],
                            scalar=1.0,
                            in1=xt[:, g, :],
                            op0=mybir.AluOpType.mult,
                            op1=mybir.AluOpType.mult,
                            accum_out=racc[:, j : j + 1],
                        )
                    else:
                        nc.scalar.activation(
                            xt[:, g, :],
                            xt[:, g, :],
                            mybir.ActivationFunctionType.Square,
                            accum_out=racc[:, j : j + 1],
                        )

            nc.sync.dma_start(p_d[:], racc[:])

    nc.compile()
    return nc


def _get_nc(kind: str):
    if kind not in _CACHE:
        _CACHE[kind] = (
            _build_gather_nc() if kind == "gather" else _build_stream_nc()
        )
    return _CACHE[kind]


def _hists(mask_id, unmask_id):
    rows = np.arange(B)[:, None]
    cm = np.zeros((B, S), np.float64)
    np.add.at(cm, (rows, mask_id.astype(np.int64)), 1.0)
    cu = np.zeros((B, S), np.float64)
    np.add.at(cu, (rows, unmask_id.astype(np.int64)), 1.0)
    return cm, cu


def _decompose(ref_c):
    """Runs of consecutive referenced rows -> exact {8,4,2,1} window cover.
    Returns {w: list of start rows} or None if any class overflows CAPS."""
    d = np.diff(np.concatenate([[0], ref_c.astype(np.int8), [0]]))
    starts = np.nonzero(d == 1)[0]
    ends = np.nonzero(d == -1)[0]
    by_w = {w: [] for w, _ in CLASSES}
    for s, e in zip(starts, ends):
        pos, L = int(s), int(e - s)
        for w, _ in CLASSES:
            q, L = divmod(L, w)
            for _ in range(q):
                by_w[w].append(pos)
                pos += w
    for w, _ in CLASSES:
        if len(by_w[w]) > CAPS[w]:
            if _os.environ.get("K_TRUNC"):   # dev: truncate instead of fallback
                by_w[w] = by_w[w][: CAPS[w]]
            else:
                return None
    return by_w


def _gather_maps(x, y, w_full):
    """Per-core input maps + weight matrices for the gather kernel.
    Returns None if any core's window classes overflow capacity."""
    maps, wmats = [], []
    for c in range(N_CORES):
        w_c = w_full[c * R:(c + 1) * R]
        by_w = _decompose(w_c > 0)
        if by_w is None:
            return None, None
        idx_blocks = []
        wm = np.zeros((128, NCOL), np.float64)
        rcol = 0
        for w, nch in CLASSES:
            cs = ROWS_PER_CHUNK // w
            # pad with row 0 (always-valid window, weight 0): every slot is
            # gathered, so num_idxs_reg == num_idxs holds and no slot ever
            # holds stale SBUF garbage
            arr = np.zeros(nch * cs, np.int64)
            arr[: len(by_w[w])] = by_w[w]
            for j in range(nch):
                blk = arr[j * cs:(j + 1) * cs].reshape(cs // 16, 16).T
                idx_blocks.append(np.tile(blk, (8, 1)).astype(np.int16))
            g = np.arange(nch * cs)
            valid = g < len(by_w[w])
            jj, ii = g // cs, g % cs
            pp, cc = ii % 128, ii // 128
            for r in range(w):
                col = rcol + jj * (ROWS_PER_CHUNK // 128) + cc * w + r
                wm[pp[valid], col[valid]] = w_c[arr[valid] + r]
            rcol += nch * (ROWS_PER_CHUNK // 128)
        maps.append({
            "x": x[c * R:(c + 1) * R],
            "y": y[c * R:(c + 1) * R],
            "idx": np.ascontiguousarray(np.concatenate(idx_blocks, axis=1)),
        })
        wmats.append(wm)
    return maps, wmats


def _stream_maps(x, y, w_full):
    maps, wmats = [], []
    for c in range(N_CORES):
        w_c = w_full[c * R:(c + 1) * R]
        maps.append({"x": x[c * R:(c + 1) * R], "y": y[c * R:(c + 1) * R]})
        wmats.append(
            w_c.reshape(N_TILES_FULL, GROUPS, 128)
            .transpose(2, 0, 1)
            .reshape(128, N_TILES_FULL * GROUPS)
        )
    return maps, wmats


def _in_maps(outputs, orig_image, mask_id, unmask_id, force_stream: bool = False):
    cm, cu = _hists(np.asarray(mask_id), np.asarray(unmask_id))
    w = (cm / (B * NM * D) + ALPHA * cu / (B * NU * D)).reshape(B * S)  # f64

    x = np.ascontiguousarray(np.asarray(outputs, dtype=np.float32)).reshape(B * S, D)
    y = np.ascontiguousarray(np.asarray(orig_image, dtype=np.float32)).reshape(B * S, D)

    if not force_stream:
        maps, wmats = _gather_maps(x, y, w)
        if maps is not None:
            return maps, "gather", wmats
    maps, wmats = _stream_maps(x, y, w)
    return maps, "stream", wmats


def _run(inputs: dict, trace: bool = False, force_stream: bool = False, **kw):
    from concourse.bass_utils import run_bass_kernel_spmd

    maps, kind, wmats = _in_maps(**inputs, force_stream=force_stream)
    nc = _get_nc(kind)
    res = run_bass_kernel_spmd(nc, maps, list(range(N_CORES)), trace=trace, **kw)
    total = np.float64(0.0)
    for c in range(N_CORES):
        racc = np.asarray(res.results[c]["racc_out"], dtype=np.float64)
        wm = wmats[c]
        m = wm != 0
        total += (racc[m] * wm[m]).sum()
    return np.asarray(total, dtype=np.float32), res


def kernel(outputs, orig_image, mask_id, unmask_id):
    outputs = np.asarray(outputs)
    orig_image = np.asarray(orig_image)
    mask_id = np.asarray(mask_id)
    unmask_id = np.asarray(unmask_id)
    assert outputs.shape == (B, S, D), outputs.shape
    assert orig_image.shape == (B, S, D), orig_image.shape
    assert mask_id.shape == (B, NM), mask_id.shape
    assert unmask_id.shape == (B, NU), unmask_id.shape
    out, _ = _run(
        {
            "outputs": outputs,
            "orig_image": orig_image,
            "mask_id": mask_id,
            "unmask_id": unmask_id,
        }
    )
    return out


# revision 13
# speedup vs baseline: 1.2039x; 1.1293x over previous
"""Trainium2 Bass kernel for nn_MAE_CalcLoss_Raw (masked MSE loss).

reference math:
    masked   = mean_b[ mean_{i,d} (outputs[b, mask_id[b,i], d]   - orig[b, mask_id[b,i], d])^2 ]
    unmasked = mean_b[ mean_{i,d} (outputs[b, unmask_id[b,i], d] - orig[b, unmask_id[b,i], d])^2 ]
    loss = masked + 0.1 * unmasked

Rewrite: gathering rows by index (with repeats) is a weighted sum over
referenced (b, s) rows.  With cnt_m[b,s] = #occurrences of s in
mask_id[b], cnt_u likewise:

    loss = sum_{b,s} w[b,s] * ||outputs[b,s,:] - orig[b,s,:]||^2
    w[b,s] = cnt_m[b,s]/(B*Nm*D) + ALPHA*cnt_u[b,s]/(B*Nu*D)

Only ~63% of rows are referenced (2048 draws with replacement from 2048
rows -> 1-1/e distinct), so instead of streaming both tensors in full
(HBM-bound at ~358 GB/s/core = ~187 us) the kernel gathers just the
referenced rows (~42 MB/core -> ~120 us floor).

The gather uses the InstDMAGatherAnt custom GPSIMD instruction.  Its
Q7 descriptor generation costs ~8.8 ns/index and is serialized on the
Pool engine, so per-row gathers (~10.4k rows x 2 tensors/core) would be
Pool-bound at ~185 us.  Runs of consecutive referenced rows are instead
decomposed exactly into windows of {8,4,2,1} rows (one descriptor per
window, elem_step=512 < elem_size allows windows at arbitrary row
offsets via a manually-built overlapping access pattern).  ~5.05k
windows/tensor/core -> Pool ~90 us, hidden under the ~120 us DMA.

Per chunk (1024 gathered rows, 2 MB/tensor): gather x, gather y (Pool),
subtract in place (DVE), then 8 square+per-row-accumulate ops (6 on
ACT, 2 on DVE) into a [128, 104] accumulator DMA'd out raw; the host
applies the per-row histogram weights in float64 (pad slots are masked
out by weight==0).  Data-parallel over B: 8 samples per core.

If a window class overflows its compiled capacity (won't happen for
this input distribution; margins are >5 sigma), the kernel falls back
to the previous full-streaming variant which is always correct.
"""

import numpy as np

ALPHA = 0.1
B, S, D = 64, 2048, 512
NM, NU = 1536, 512
N_CORES = 8
BPC = B // N_CORES            # samples per core
R = BPC * S                   # rows per core = 16384

# --- gather-kernel geometry ---
# (window_rows, [chunk slot counts]); caps are max-observed-per-core + >5
# sigma margin (max seen: w1 2391, w2 1764, w4 875, w8 162).  Pool-heavy
# classes (many descriptors per byte) go first so the kernel tail is small
# and DMA-bound.  Chunk slot counts must be multiples of 16; the last chunk
# of each class is small to shorten the pipeline tail.
CLASSES = [
    (1, [1024, 1024, 400]),          # cap 2448
    (2, [512, 512, 512, 288]),       # cap 1824
    (4, [256, 256, 256, 144]),       # cap 912
    (8, [128, 48]),                  # cap 176
]
import os as _os
if _os.environ.get("K_CLASSES"):
    CLASSES = [
        (int(p.split(":")[0]), [int(x) for x in p.split(":")[1].split("/")])
        for p in _os.environ["K_CLASSES"].split(",")
    ]
def _cdiv(a, b):
    return -(-a // b)


# per-chunk racc columns = ceil(cs/128) * w
NCOL = sum(_cdiv(cs, 128) * w for w, csl in CLASSES for cs in csl)
IDXCOL = sum(cs // 16 for _, csl in CLASSES for cs in csl)
CAPS = {w: sum(csl) for w, csl in CLASSES}
ACT_FRAC = 0.75               # fraction of per-chunk accum columns on ACT

# --- streaming-kernel geometry (fallback) ---
GROUPS = 8                    # 128-row groups per tile
TILE_ROWS = GROUPS * 128      # 1024 rows per tile (2 MB per tensor)
N_TILES_FULL = R // TILE_ROWS          # 16

_CACHE: dict = {}


def _build_gather_nc():
    import concourse.bacc as bacc
    import concourse.tile as tile
    import concourse.mybir as mybir
    import bass_rust

    f32 = mybir.dt.float32
    i16 = mybir.dt.int16

    nc = bacc.Bacc(
        "TRN2",
        target_bir_lowering=False,
        debug=False,
        enable_asserts=False,
        num_devices=N_CORES,
    )
    x_d = nc.dram_tensor("x", [R, D], f32, kind="ExternalInput").ap()
    y_d = nc.dram_tensor("y", [R, D], f32, kind="ExternalInput").ap()
    idx_d = nc.dram_tensor("idx", [128, IDXCOL], i16, kind="ExternalInput").ap()
    p_d = nc.dram_tensor("racc_out", [128, NCOL], f32, kind="ExternalOutput").ap()

    # Overlapping window views: row-stride 512 elems, window length w*512.
    def win_view(base, w):
        if w == 1:
            return base
        v = base.copy()
        v.ap = bass_rust.VecI64Pair([[D, R - w + 1], [1, w * D]])
        return v

    xv = {w: win_view(x_d, w) for w, _ in CLASSES}
    yv = {w: win_view(y_d, w) for w, _ in CLASSES}

    with tile.TileContext(nc) as tc:
        with (
            tc.tile_pool(name="io", bufs=int(_os.environ.get("K_BUFS", "6"))) as io,
            tc.tile_pool(name="acc", bufs=1) as acc,
        ):
            idx_sb = acc.tile([128, IDXCOL], i16, tag="idx")
            nc.sync.dma_start(idx_sb[:], idx_d[:])
            racc = acc.tile([128, NCOL], f32, tag="racc")

            icol = 0
            rcol = 0
            for w, csl in CLASSES:
                for cs in csl:
                    ccols = _cdiv(cs, 128)     # tile columns
                    icols = cs // 16           # idx columns this chunk
                    xt = io.tile([128, ccols, w * D], f32, tag="x")
                    yt = io.tile([128, ccols, w * D], f32, tag="y")
                    ixap = idx_sb[:, icol:icol + icols]
                    step = None if w == 1 else D
                    nc.gpsimd.dma_gather(
                        xt[:], xv[w], ixap, cs, cs, w * D, elem_step=step)
                    nc.gpsimd.dma_gather(
                        yt[:], yv[w], ixap, cs, cs, w * D, elem_step=step)
                    nc.vector.tensor_sub(xt[:], xt[:], yt[:])
                    ncols = ccols * w          # racc columns this chunk
                    nact = round(ACT_FRAC * ncols)
                    for g in range(ncols):
                        c, r = divmod(g, w)
                        src = xt[:, c, r * D:(r + 1) * D]
                        col = racc[:, rcol + g:rcol + g + 1]
                        if g < nact:
                            nc.scalar.activation(
                                src, src,
                                mybir.ActivationFunctionType.Square,
                                accum_out=col)
                        else:
                            nc.vector.scalar_tensor_tensor(
                                out=src, in0=src, scalar=1.0, in1=src,
                                op0=mybir.AluOpType.mult,
                                op1=mybir.AluOpType.mult,
                                accum_out=col)
                    icol += icols
                    rcol += ncols

            nc.sync.dma_start(p_d[:], racc[:])

    nc.compile()
    return nc


def _build_stream_nc():
    import concourse.bacc as bacc
    import concourse.bass as bass
    import concourse.tile as tile
    import concourse.mybir as mybir

    f32 = mybir.dt.float32
    ncol = N_TILES_FULL * GROUPS
    nc = bacc.Bacc(
        "TRN2",
        target_bir_lowering=False,
        debug=False,
        enable_asserts=False,
        num_devices=N_CORES,
    )
    x_d = nc.dram_tensor("x", [R, D], f32, kind="ExternalInput").ap()
    y_d = nc.dram_tensor("y", [R, D], f32, kind="ExternalInput").ap()
    p_d = nc.dram_tensor("racc_out", [128, ncol], f32, kind="ExternalOutput").ap()

    with tile.TileContext(nc) as tc:
        with (
            tc.tile_pool(name="io", bufs=4) as io,
            tc.tile_pool(name="acc", bufs=1) as acc,
        ):
            racc = acc.tile([128, ncol], f32, tag="racc")

            HG = GROUPS // 2  # half-tile: 4 groups, 1 MB per tensor
            n_halves = 2 * N_TILES_FULL
            for h in range(n_halves):
                if h == n_halves - 1:
                    # final half-tile in single-group chunks: shortens the
                    # compute tail after the last DMA lands
                    for g in range(HG):
                        j = h * HG + g
                        xg = io.tile([128, 1, D], f32, tag="xf")
                        nc.sync.dma_start(
                            xg[:],
                            x_d[bass.ts(j, 128), :].rearrange(
                                "(g p) d -> p g d", g=1, p=128
                            ),
                        )
                        yg = io.tile([128, 1, D], f32, tag="yf")
                        nc.sync.dma_start(
                            yg[:],
                            y_d[bass.ts(j, 128), :].rearrange(
                                "(g p) d -> p g d", g=1, p=128
                            ),
                        )
                        nc.vector.tensor_sub(xg[:], xg[:], yg[:])
                        if g == HG - 1:
                            nc.vector.scalar_tensor_tensor(
                                out=xg[:, 0, :],
                                in0=xg[:, 0, :],
                                scalar=1.0,
                                in1=xg[:, 0, :],
                                op0=mybir.AluOpType.mult,
                                op1=mybir.AluOpType.mult,
                                accum_out=racc[:, j : j + 1],
                            )
                        else:
                            nc.scalar.activation(
                                xg[:, 0, :],
                                xg[:, 0, :],
                                mybir.ActivationFunctionType.Square,
                                accum_out=racc[:, j : j + 1],
                            )
                    continue
                xt = io.tile([128, HG, D], f32, tag="x")
                yt = io.tile([128, HG, D], f32, tag="y")
                nc.sync.dma_start(
                    xt[:],
                    x_d[bass.ts(h, HG * 128), :].rearrange(
                        "(g p) d -> p g d", g=HG, p=128
                    ),
                )
                nc.sync.dma_start(
                    yt[:],
                    y_d[bass.ts(h, HG * 128), :].rearrange(
                        "(g p) d -> p g d", g=HG, p=128
                    ),
                )
                # diff in place on DVE
                nc.vector.tensor_sub(xt[:], xt[:], yt[:])
                # square + per-row accumulate: 3 groups on ACT, 1 on DVE
                for g in range(HG):
                    j = h * HG + g
                    if g == HG - 1:
                        nc.vector.scalar_tensor_tensor(
                            out=xt[:, g, :],
                            in0=xt[:, g, :],
                            scalar=1.0,
                            in1=xt[:, g, :],
                            op0=mybir.AluOpType.mult,
                            op1=mybir.AluOpType.mult,
                            accum_out=racc[:, j : j + 1],
                        )
                    else:
                        nc.scalar.activation(
                            xt[:, g, :],
                            xt[:, g, :],
                            mybir.ActivationFunctionType.Square,
                            accum_out=racc[:, j : j + 1],
                        )

            nc.sync.dma_start(p_d[:], racc[:])

    nc.compile()
    return nc


def _get_nc(kind: str):
    if kind not in _CACHE:
        _CACHE[kind] = (
            _build_gather_nc() if kind == "gather" else _build_stream_nc()
        )
    return _CACHE[kind]


def _hists(mask_id, unmask_id):
    rows = np.arange(B)[:, None]
    cm = np.zeros((B, S), np.float64)
    np.add.at(cm, (rows, mask_id.astype(np.int64)), 1.0)
    cu = np.zeros((B, S), np.float64)
    np.add.at(cu, (rows, unmask_id.astype(np.int64)), 1.0)
    return cm, cu


def _decompose(ref_c):
    """Runs of consecutive referenced rows -> exact {8,4,2,1} window cover.
    Returns {w: list of start rows} or None if any class overflows CAPS."""
    d = np.diff(np.concatenate([[0], ref_c.astype(np.int8), [0]]))
    starts = np.nonzero(d == 1)[0]
    ends = np.nonzero(d == -1)[0]
    by_w = {w: [] for w, _ in CLASSES}
    for s, e in zip(starts, ends):
        pos, L = int(s), int(e - s)
        for w in sorted(by_w, reverse=True):
            q, L = divmod(L, w)
            for _ in range(q):
                by_w[w].append(pos)
                pos += w
    for w, _ in CLASSES:
        if len(by_w[w]) > CAPS[w]:
            if _os.environ.get("K_TRUNC"):   # dev: truncate instead of fallback
                by_w[w] = by_w[w][: CAPS[w]]
            else:
                return None
    return by_w


def _gather_maps(x, y, w_full):
    """Per-core input maps + weight matrices for the gather kernel.
    Returns None if any core's window classes overflow capacity."""
    maps, wmats = [], []
    for c in range(N_CORES):
        w_c = w_full[c * R:(c + 1) * R]
        by_w = _decompose(w_c > 0)
        if by_w is None:
            return None, None
        idx_blocks = []
        wm = np.zeros((128, NCOL), np.float64)
        rcol = 0
        for w, csl in CLASSES:
            # pad with row 0 (always-valid window, weight 0): every slot is
            # gathered, so num_idxs_reg == num_idxs holds and no slot ever
            # holds stale SBUF garbage
            n_w = len(by_w[w])
            arr_all = np.zeros(sum(csl), np.int64)
            arr_all[:n_w] = by_w[w]
            off = 0
            for cs in csl:
                arr = arr_all[off:off + cs]
                blk = arr.reshape(cs // 16, 16).T
                idx_blocks.append(np.tile(blk, (8, 1)).astype(np.int16))
                i = np.arange(cs)
                valid = (off + i) < n_w
                pp, cc = i % 128, i // 128
                for r in range(w):
                    col = rcol + cc * w + r
                    wm[pp[valid], col[valid]] = w_c[arr[valid] + r]
                rcol += _cdiv(cs, 128) * w
                off += cs
        maps.append({
            "x": x[c * R:(c + 1) * R],
            "y": y[c * R:(c + 1) * R],
            "idx": np.ascontiguousarray(np.concatenate(idx_blocks, axis=1)),
        })
        wmats.append(wm)
    return maps, wmats


def _stream_maps(x, y, w_full):
    maps, wmats = [], []
    for c in range(N_CORES):
        w_c = w_full[c * R:(c + 1) * R]
        maps.append({"x": x[c * R:(c + 1) * R], "y": y[c * R:(c + 1) * R]})
        wmats.append(
            w_c.reshape(N_TILES_FULL, GROUPS, 128)
            .transpose(2, 0, 1)
            .reshape(128, N_TILES_FULL * GROUPS)
        )
    return maps, wmats


def _in_maps(outputs, orig_image, mask_id, unmask_id, force_stream: bool = False):
    cm, cu = _hists(np.asarray(mask_id), np.asarray(unmask_id))
    w = (cm / (B * NM * D) + ALPHA * cu / (B * NU * D)).reshape(B * S)  # f64

    x = np.ascontiguousarray(np.asarray(outputs, dtype=np.float32)).reshape(B * S, D)
    y = np.ascontiguousarray(np.asarray(orig_image, dtype=np.float32)).reshape(B * S, D)

    if not force_stream:
        maps, wmats = _gather_maps(x, y, w)
        if maps is not None:
            return maps, "gather", wmats
    maps, wmats = _stream_maps(x, y, w)
    return maps, "stream", wmats


def _run(inputs: dict, trace: bool = False, force_stream: bool = False, **kw):
    from concourse.bass_utils import run_bass_kernel_spmd

    maps, kind, wmats = _in_maps(**inputs, force_stream=force_stream)
    nc = _get_nc(kind)
    res = run_bass_kernel_spmd(nc, maps, list(range(N_CORES)), trace=trace, **kw)
    total = np.float64(0.0)
    for c in range(N_CORES):
        racc = np.asarray(res.results[c]["racc_out"], dtype=np.float64)
        wm = wmats[c]
        m = wm != 0
        total += (racc[m] * wm[m]).sum()
    return np.asarray(total, dtype=np.float32), res


def kernel(outputs, orig_image, mask_id, unmask_id):
    outputs = np.asarray(outputs)
    orig_image = np.asarray(orig_image)
    mask_id = np.asarray(mask_id)
    unmask_id = np.asarray(unmask_id)
    assert outputs.shape == (B, S, D), outputs.shape
    assert orig_image.shape == (B, S, D), orig_image.shape
    assert mask_id.shape == (B, NM), mask_id.shape
    assert unmask_id.shape == (B, NU), unmask_id.shape
    out, _ = _run(
        {
            "outputs": outputs,
            "orig_image": orig_image,
            "mask_id": mask_id,
            "unmask_id": unmask_id,
        }
    )
    return out


# revision 15
# speedup vs baseline: 1.3793x; 1.1457x over previous
"""Trainium2 Bass kernel for nn_MAE_CalcLoss_Raw (masked MSE loss).

reference math:
    masked   = mean_b[ mean_{i,d} (outputs[b, mask_id[b,i], d]   - orig[b, mask_id[b,i], d])^2 ]
    unmasked = mean_b[ mean_{i,d} (outputs[b, unmask_id[b,i], d] - orig[b, unmask_id[b,i], d])^2 ]
    loss = masked + 0.1 * unmasked

Rewrite: gathering rows by index (with repeats) is a weighted sum over
referenced (b, s) rows.  With cnt_m[b,s] = #occurrences of s in
mask_id[b], cnt_u likewise:

    loss = sum_{b,s} w[b,s] * ||outputs[b,s,:] - orig[b,s,:]||^2
    w[b,s] = cnt_m[b,s]/(B*Nm*D) + ALPHA*cnt_u[b,s]/(B*Nu*D)

Only ~63% of rows are referenced (2048 draws with replacement from 2048
rows -> 1-1/e distinct), so instead of streaming both tensors in full
(HBM-bound at ~358 GB/s/core = ~187 us) the kernel gathers just the
referenced rows (~42 MB/core -> ~120 us floor).

The gather uses the InstDMAGatherAnt custom GPSIMD instruction.  Its
Q7 descriptor generation costs ~8.8 ns/index and is serialized on the
Pool engine, so per-row gathers (~10.4k rows x 2 tensors/core) would be
Pool-bound at ~185 us.  Runs of consecutive referenced rows are instead
decomposed exactly into windows of {8,4,2,1} rows (one descriptor per
window, elem_step=512 < elem_size allows windows at arbitrary row
offsets via a manually-built overlapping access pattern).  ~5.05k
windows/tensor/core -> Pool ~90 us, hidden under the ~120 us DMA.

Per chunk (1024 gathered rows, 2 MB/tensor): gather x, gather y (Pool),
subtract in place (DVE), then 8 square+per-row-accumulate ops (6 on
ACT, 2 on DVE) into a [128, 104] accumulator DMA'd out raw; the host
applies the per-row histogram weights in float64 (pad slots are masked
out by weight==0).  Data-parallel over B: 8 samples per core.

If a window class overflows its compiled capacity (won't happen for
this input distribution; margins are >5 sigma), the kernel falls back
to the previous full-streaming variant which is always correct.
"""

import numpy as np

ALPHA = 0.1
B, S, D = 64, 2048, 512
NM, NU = 1536, 512
N_CORES = 8
BPC = B // N_CORES            # samples per core
R = BPC * S                   # rows per core = 16384

# --- gather-kernel geometry ---
# (window_rows, [chunk slot counts]); caps are max-observed-per-core + >5
# sigma margin (max seen: w1 2391, w2 1764, w4 875, w8 162).  Pool-heavy
# classes (many descriptors per byte) go first so the kernel tail is small
# and DMA-bound.  Chunk slot counts must be multiples of 16; the last chunk
# of each class is small to shorten the pipeline tail.
CLASSES = [
    (1, [1024, 1024, 400]),          # cap 2448
    (2, [512, 512, 512, 288]),       # cap 1824
    (4, [256, 256, 256, 144]),       # cap 912
    (8, [128, 48]),                  # cap 176
]
import os as _os
if _os.environ.get("K_CLASSES"):
    CLASSES = [
        (int(p.split(":")[0]), [int(x) for x in p.split(":")[1].split("/")])
        for p in _os.environ["K_CLASSES"].split(",")
    ]
def _cdiv(a, b):
    return -(-a // b)


# per-chunk racc columns = ceil(cs/128) * w
NCOL = sum(_cdiv(cs, 128) * w for w, csl in CLASSES for cs in csl)
IDXCOL = sum(cs // 16 for _, csl in CLASSES for cs in csl)
CAPS = {w: sum(csl) for w, csl in CLASSES}
ACT_FRAC = 0.75               # fraction of per-chunk accum columns on ACT

# --- streaming-kernel geometry (fallback) ---
GROUPS = 8                    # 128-row groups per tile
TILE_ROWS = GROUPS * 128      # 1024 rows per tile (2 MB per tensor)
N_TILES_FULL = R // TILE_ROWS          # 16

_CACHE: dict = {}


def _build_gather_nc():
    import concourse.bacc as bacc
    import concourse.tile as tile
    import concourse.mybir as mybir
    import bass_rust

    f32 = mybir.dt.float32
    i16 = mybir.dt.int16

    nq = int(_os.environ.get("K_NQ", "4"))
    nc = bacc.Bacc(
        "TRN2",
        target_bir_lowering=False,
        debug=False,
        enable_asserts=False,
        num_devices=N_CORES,
        num_swdge_queues=nq,
    )
    x_d = nc.dram_tensor("x", [R, D], f32, kind="ExternalInput").ap()
    y_d = nc.dram_tensor("y", [R, D], f32, kind="ExternalInput").ap()
    idx_d = nc.dram_tensor("idx", [128, IDXCOL], i16, kind="ExternalInput").ap()
    p_d = nc.dram_tensor("racc_out", [128, NCOL], f32, kind="ExternalOutput").ap()

    # Overlapping window views: row-stride 512 elems, window length w*512.
    def win_view(base, w):
        if w == 1:
            return base
        v = base.copy()
        v.ap = bass_rust.VecI64Pair([[D, R - w + 1], [1, w * D]])
        return v

    xv = {w: win_view(x_d, w) for w, _ in CLASSES}
    yv = {w: win_view(y_d, w) for w, _ in CLASSES}

    with tile.TileContext(nc) as tc:
        with (
            tc.tile_pool(name="io", bufs=int(_os.environ.get("K_BUFS", "6"))) as io,
            tc.tile_pool(name="acc", bufs=1) as acc,
        ):
            idx_sb = acc.tile([128, IDXCOL], i16, tag="idx")
            nc.sync.dma_start(idx_sb[:], idx_d[:])
            racc = acc.tile([128, NCOL], f32, tag="racc")

            icol = 0
            rcol = 0
            gidx = 0
            for w, csl in CLASSES:
                for cs in csl:
                    ccols = _cdiv(cs, 128)     # tile columns
                    icols = cs // 16           # idx columns this chunk
                    xt = io.tile([128, ccols, w * D], f32, tag="x")
                    yt = io.tile([128, ccols, w * D], f32, tag="y")
                    ixap = idx_sb[:, icol:icol + icols]
                    step = None if w == 1 else D
                    nc.gpsimd.dma_gather(
                        xt[:], xv[w], ixap, cs, cs, w * D, elem_step=step,
                        queue_num=gidx % nq)
                    gidx += 1
                    nc.gpsimd.dma_gather(
                        yt[:], yv[w], ixap, cs, cs, w * D, elem_step=step,
                        queue_num=gidx % nq)
                    gidx += 1
                    nc.vector.tensor_sub(xt[:], xt[:], yt[:])
                    ncols = ccols * w          # racc columns this chunk
                    nact = round(ACT_FRAC * ncols)
                    for g in range(ncols):
                        c, r = divmod(g, w)
                        src = xt[:, c, r * D:(r + 1) * D]
                        col = racc[:, rcol + g:rcol + g + 1]
                        if g < nact:
                            nc.scalar.activation(
                                src, src,
                                mybir.ActivationFunctionType.Square,
                                accum_out=col)
                        else:
                            nc.vector.scalar_tensor_tensor(
                                out=src, in0=src, scalar=1.0, in1=src,
                                op0=mybir.AluOpType.mult,
                                op1=mybir.AluOpType.mult,
                                accum_out=col)
                    icol += icols
                    rcol += ncols

            nc.sync.dma_start(p_d[:], racc[:])

    nc.compile()
    return nc


def _build_stream_nc():
    import concourse.bacc as bacc
    import concourse.bass as bass
    import concourse.tile as tile
    import concourse.mybir as mybir

    f32 = mybir.dt.float32
    ncol = N_TILES_FULL * GROUPS
    nc = bacc.Bacc(
        "TRN2",
        target_bir_lowering=False,
        debug=False,
        enable_asserts=False,
        num_devices=N_CORES,
    )
    x_d = nc.dram_tensor("x", [R, D], f32, kind="ExternalInput").ap()
    y_d = nc.dram_tensor("y", [R, D], f32, kind="ExternalInput").ap()
    p_d = nc.dram_tensor("racc_out", [128, ncol], f32, kind="ExternalOutput").ap()

    with tile.TileContext(nc) as tc:
        with (
            tc.tile_pool(name="io", bufs=4) as io,
            tc.tile_pool(name="acc", bufs=1) as acc,
        ):
            racc = acc.tile([128, ncol], f32, tag="racc")

            HG = GROUPS // 2  # half-tile: 4 groups, 1 MB per tensor
            n_halves = 2 * N_TILES_FULL
            for h in range(n_halves):
                if h == n_halves - 1:
                    # final half-tile in single-group chunks: shortens the
                    # compute tail after the last DMA lands
                    for g in range(HG):
                        j = h * HG + g
                        xg = io.tile([128, 1, D], f32, tag="xf")
                        nc.sync.dma_start(
                            xg[:],
                            x_d[bass.ts(j, 128), :].rearrange(
                                "(g p) d -> p g d", g=1, p=128
                            ),
                        )
                        yg = io.tile([128, 1, D], f32, tag="yf")
                        nc.sync.dma_start(
                            yg[:],
                            y_d[bass.ts(j, 128), :].rearrange(
                                "(g p) d -> p g d", g=1, p=128
                            ),
                        )
                        nc.vector.tensor_sub(xg[:], xg[:], yg[:])
                        if g == HG - 1:
                            nc.vector.scalar_tensor_tensor(
                                out=xg[:, 0, :],
                                in0=xg[:, 0, :],
                                scalar=1.0,
                                in1=xg[:, 0, :],
                                op0=mybir.AluOpType.mult,
                                op1=mybir.AluOpType.mult,
                                accum_out=racc[:, j : j + 1],
                            )
                        else:
                            nc.scalar.activation(
                                xg[:, 0, :],
                                xg[:, 0, :],
                                mybir.ActivationFunctionType.Square,
                                accum_out=racc[:, j : j + 1],
                            )
                    continue
                xt = io.tile([128, HG, D], f32, tag="x")
                yt = io.tile([128, HG, D], f32, tag="y")
                nc.sync.dma_start(
                    xt[:],
                    x_d[bass.ts(h, HG * 128), :].rearrange(
                        "(g p) d -> p g d", g=HG, p=128
                    ),
                )
                nc.sync.dma_start(
                    yt[:],
                    y_d[bass.ts(h, HG * 128), :].rearrange(
                        "(g p) d -> p g d", g=HG, p=128
                    ),
                )
                # diff in place on DVE
                nc.vector.tensor_sub(xt[:], xt[:], yt[:])
                # square + per-row accumulate: 3 groups on ACT, 1 on DVE
                for g in range(HG):
                    j = h * HG + g
                    if g == HG - 1:
                        nc.vector.scalar_tensor_tensor(
                            out=xt[:, g, :],
                            in0=xt[:, g, :],
                            scalar=1.0,
                            in1=xt[:, g, :],
                            op0=mybir.AluOpType.mult,
                            op1=mybir.AluOpType.mult,
                            accum_out=racc[:, j : j + 1],
                        )
                    else:
                        nc.scalar.activation(
                            xt[:, g, :],
                            xt[:, g, :],
                            mybir.ActivationFunctionType.Square,
                            accum_out=racc[:, j : j + 1],
                        )

            nc.sync.dma_start(p_d[:], racc[:])

    nc.compile()
    return nc


def _get_nc(kind: str):
    if kind not in _CACHE:
        _CACHE[kind] = (
            _build_gather_nc() if kind == "gather" else _build_stream_nc()
        )
    return _CACHE[kind]


def _hists(mask_id, unmask_id):
    rows = np.arange(B)[:, None]
    cm = np.zeros((B, S), np.float64)
    np.add.at(cm, (rows, mask_id.astype(np.int64)), 1.0)
    cu = np.zeros((B, S), np.float64)
    np.add.at(cu, (rows, unmask_id.astype(np.int64)), 1.0)
    return cm, cu


def _decompose(ref_c):
    """Runs of consecutive referenced rows -> exact {8,4,2,1} window cover.
    Returns {w: list of start rows} or None if any class overflows CAPS."""
    d = np.diff(np.concatenate([[0], ref_c.astype(np.int8), [0]]))
    starts = np.nonzero(d == 1)[0]
    ends = np.nonzero(d == -1)[0]
    by_w = {w: [] for w, _ in CLASSES}
    for s, e in zip(starts, ends):
        pos, L = int(s), int(e - s)
        for w in sorted(by_w, reverse=True):
            q, L = divmod(L, w)
            for _ in range(q):
                by_w[w].append(pos)
                pos += w
    for w, _ in CLASSES:
        if len(by_w[w]) > CAPS[w]:
            if _os.environ.get("K_TRUNC"):   # dev: truncate instead of fallback
                by_w[w] = by_w[w][: CAPS[w]]
            else:
                return None
    return by_w


def _gather_maps(x, y, w_full):
    """Per-core input maps + weight matrices for the gather kernel.
    Returns None if any core's window classes overflow capacity."""
    maps, wmats = [], []
    for c in range(N_CORES):
        w_c = w_full[c * R:(c + 1) * R]
        by_w = _decompose(w_c > 0)
        if by_w is None:
            return None, None
        idx_blocks = []
        wm = np.zeros((128, NCOL), np.float64)
        rcol = 0
        for w, csl in CLASSES:
            # pad with row 0 (always-valid window, weight 0): every slot is
            # gathered, so num_idxs_reg == num_idxs holds and no slot ever
            # holds stale SBUF garbage
            n_w = len(by_w[w])
            arr_all = np.zeros(sum(csl), np.int64)
            arr_all[:n_w] = by_w[w]
            off = 0
            for cs in csl:
                arr = arr_all[off:off + cs]
                blk = arr.reshape(cs // 16, 16).T
                idx_blocks.append(np.tile(blk, (8, 1)).astype(np.int16))
                i = np.arange(cs)
                valid = (off + i) < n_w
                pp, cc = i % 128, i // 128
                for r in range(w):
                    col = rcol + cc * w + r
                    wm[pp[valid], col[valid]] = w_c[arr[valid] + r]
                rcol += _cdiv(cs, 128) * w
                off += cs
        maps.append({
            "x": x[c * R:(c + 1) * R],
            "y": y[c * R:(c + 1) * R],
            "idx": np.ascontiguousarray(np.concatenate(idx_blocks, axis=1)),
        })
        wmats.append(wm)
    return maps, wmats


def _stream_maps(x, y, w_full):
    maps, wmats = [], []
    for c in range(N_CORES):
        w_c = w_full[c * R:(c + 1) * R]
        maps.append({"x": x[c * R:(c + 1) * R], "y": y[c * R:(c + 1) * R]})
        wmats.append(
            w_c.reshape(N_TILES_FULL, GROUPS, 128)
            .transpose(2, 0, 1)
            .reshape(128, N_TILES_FULL * GROUPS)
        )
    return maps, wmats


def _in_maps(outputs, orig_image, mask_id, unmask_id, force_stream: bool = False):
    cm, cu = _hists(np.asarray(mask_id), np.asarray(unmask_id))
    w = (cm / (B * NM * D) + ALPHA * cu / (B * NU * D)).reshape(B * S)  # f64

    x = np.ascontiguousarray(np.asarray(outputs, dtype=np.float32)).reshape(B * S, D)
    y = np.ascontiguousarray(np.asarray(orig_image, dtype=np.float32)).reshape(B * S, D)

    if not force_stream:
        maps, wmats = _gather_maps(x, y, w)
        if maps is not None:
            return maps, "gather", wmats
    maps, wmats = _stream_maps(x, y, w)
    return maps, "stream", wmats


def _run(inputs: dict, trace: bool = False, force_stream: bool = False, **kw):
    from concourse.bass_utils import run_bass_kernel_spmd

    maps, kind, wmats = _in_maps(**inputs, force_stream=force_stream)
    nc = _get_nc(kind)
    res = run_bass_kernel_spmd(nc, maps, list(range(N_CORES)), trace=trace, **kw)
    total = np.float64(0.0)
    for c in range(N_CORES):
        racc = np.asarray(res.results[c]["racc_out"], dtype=np.float64)
        wm = wmats[c]
        m = wm != 0
        total += (racc[m] * wm[m]).sum()
    return np.asarray(total, dtype=np.float32), res


def kernel(outputs, orig_image, mask_id, unmask_id):
    outputs = np.asarray(outputs)
    orig_image = np.asarray(orig_image)
    mask_id = np.asarray(mask_id)
    unmask_id = np.asarray(unmask_id)
    assert outputs.shape == (B, S, D), outputs.shape
    assert orig_image.shape == (B, S, D), orig_image.shape
    assert mask_id.shape == (B, NM), mask_id.shape
    assert unmask_id.shape == (B, NU), unmask_id.shape
    out, _ = _run(
        {
            "outputs": outputs,
            "orig_image": orig_image,
            "mask_id": mask_id,
            "unmask_id": unmask_id,
        }
    )
    return out
